# revision 1
# baseline (speedup 1.0000x reference)
"""DTNetv0 forward kernel for 8 Trainium2 NeuronCores.

Computes, for x [B,128], W1 [511,128], b1 [511], W2 [512,1022],
leaf_actions [512] (32 leaves per each of 16 actions):

    h = x @ W1.T + b1
    z = [relu(h), relu(-h)]
    y = z @ W2.T
    pooled[b,a] = max over leaves l with action a of y[b,l]
    out = softmax(pooled, axis=-1)

Sharding: pure data parallelism — batch split 8 ways, weights replicated.

Per 512-row batch tile, on device:
    xT [128in, 512b]   PE transpose of the DMA'd x tile
    hT [512nodes,512b] linear1: 4 f32r matmuls (W1T stationary, xT moving)
    zT [1024, 512b]    Relu(h+b1) on ACT; two relu(-h-b1) chunks on ACT via
                       scale=-1, two as min(h+b1,0) on DVE against
                       host-negated W2 rows (identical math, balances the
                       engines)
    y  [128b, 512lv]   linear2 BATCH-MAJOR: per 128-batch subtile, 8
                       accumulating f32r matmuls with the z chunk as the
                       STATIONARY operand and W2T as the moving one, so y
                       lands batch-major in PSUM
    pooled [128b, 16]  leaves are host-permuted so slot s holds a leaf of
                       action s%16; the whole segment max is ONE strided DVE
                       reduce straight off each PSUM bank (no transposes, no
                       partition folds)
    out                softmax batch-major: Exp with per-partition bias=-max
                       and accum_out for the denominator, reciprocal, scale

Three-stage software pipeline in emission order: front_a (x -> z) runs two
tiles ahead of front_b (matmul2 + pooled reduce), and the softmax tail
trails one tile behind, so the z chunks PE needs are always ready and the
cost-model steady state is PE-gap-free (94% tensor-engine busy).

Matmul operands use float32r: fp32 data processed at 1 cycle/row for
512-wide moving operands (plain fp32 runs at 4 cycles/row). bf16 was
measured on HW at parity with f32r (the PE is row-rate-limited regardless
of dtype) while costing 8x accuracy, so f32r stays.
"""

import numpy as np

B, IN_DIM, N_NODES, N_LEAVES, N_ACTIONS = 131072, 128, 511, 512, 16
N_CORES = 8
B_SHARD = B // N_CORES          # 16384 rows per core
B_TILE = 512                    # batch columns per tile (one PSUM bank of fp32)
N_TILES = B_SHARD // B_TILE     # 32
NODES_P = 512                   # nodes padded 511 -> 512 (4 chunks of 128)
Z_DIM = 2 * NODES_P             # 1024 (8 chunks of 128)
DVE_ZHI = (0, 1)                # z_hi chunks produced on DVE via min-trick
# bf16 z/W2 for matmul2 was measured on HW at parity with f32r (the PE is
# row-rate-limited regardless of dtype) while costing 8x accuracy — keep f32r.
MM2_BF16 = False

_compiled = None  # traced+compiled Bass module cache (one per process)


def _build_nc(n_passes=1):
    import concourse.tile as tile
    from concourse import bacc, mybir
    from concourse.masks import make_identity
    from contextlib import ExitStack

    fp32 = mybir.dt.float32
    f32r = mybir.dt.float32r
    zdt = mybir.dt.bfloat16 if MM2_BF16 else f32r
    AF = mybir.ActivationFunctionType

    nc = bacc.Bacc()
    x_h = nc.declare_dram_parameter("x", [B_SHARD, IN_DIM], f32r, isOutput=False)
    w1t_h = nc.declare_dram_parameter("w1t", [IN_DIM, NODES_P], f32r, isOutput=False)
    b1c_h = nc.declare_dram_parameter("b1c", [128, 4], fp32, isOutput=False)
    nb1c_h = nc.declare_dram_parameter("nb1c", [128, 4], fp32, isOutput=False)
    w2t_h = nc.declare_dram_parameter("w2t", [128, 8, B_TILE], zdt, isOutput=False)
    out_h = nc.declare_dram_parameter("out", [B_SHARD, N_ACTIONS], fp32, isOutput=True)

    with tile.TileContext(nc) as tc, ExitStack() as ctx:
        consts = ctx.enter_context(tc.tile_pool(name="consts", bufs=1))
        xin = ctx.enter_context(tc.tile_pool(name="xin", bufs=3))
        xts = ctx.enter_context(tc.tile_pool(name="xts", bufs=2))
        zp = ctx.enter_context(tc.tile_pool(name="zp", bufs=3))
        sm = ctx.enter_context(tc.tile_pool(name="sm", bufs=2))
        psA = ctx.enter_context(tc.tile_pool(name="psA", bufs=2, space="PSUM"))
        psY = ctx.enter_context(tc.tile_pool(name="psY", bufs=5, space="PSUM"))

        def load_x(t):
            rows = slice(t * B_TILE, (t + 1) * B_TILE)
            x_sb = xin.tile([128, 4, IN_DIM], f32r, tag="x")
            nc.sync.dma_start(
                out=x_sb, in_=x_h[rows, :].rearrange("(s p) d -> p s d", p=128)
            )
            return x_sb

        identity = consts.tile([128, 128], fp32)
        make_identity(nc, identity)
        identity_r = consts.tile([128, 128], f32r)
        nc.vector.tensor_copy(identity_r, identity)
        # prefetch the first two x tiles before the (big) weight DMAs so the
        # first transposes are not queued behind them
        x_pre = [load_x(0), load_x(1)]
        b1_sb = consts.tile([128, 4], fp32)
        nc.sync.dma_start(out=b1_sb, in_=b1c_h[:, :])
        nb1_sb = consts.tile([128, 4], fp32)
        nc.sync.dma_start(out=nb1_sb, in_=nb1c_h[:, :])
        # weights are declared float32r in DRAM (host sends fp32 bits) and
        # DMA straight into f32r tiles — no staging/rounding copies. The 2MB
        # w2t load rides the Activation HWDGE queue so the x-loads (SP
        # queue) are not stuck behind it.
        w1t_sb = consts.tile([128, NODES_P], f32r)
        nc.sync.dma_start(out=w1t_sb, in_=w1t_h[:, :])
        w2t_sb = consts.tile([128, 8, B_TILE], zdt)
        nc.scalar.dma_start(out=w2t_sb, in_=w2t_h[:, :, :])

        def front_a(t, x_sb=None):
            rows = slice(t * B_TILE, (t + 1) * B_TILE)

            # ---- x tile (possibly prefetched) -> transpose to [in, batch] ----
            if x_sb is None:
                x_sb = load_x(t)
            xt_ps = psA.tile([128, 4, 128], f32r, tag="xt", bufs=1)
            for s in range(4):
                nc.tensor.transpose(xt_ps[:, s, :], x_sb[:, s, :], identity_r)
            xt_sb = xts.tile([128, 4, 128], f32r, tag="xt_sb")
            nc.vector.tensor_copy(xt_sb, xt_ps)
            xt_mm = xt_sb.rearrange("p s d -> p (s d)")

            # ---- linear1 + fused bias/relu into zT [128, 8, 512] ----
            z_sb = zp.tile([128, 8, B_TILE], zdt, tag="z")
            for c in range(4):
                h_ps = psA.tile([128, B_TILE], fp32, tag="h")
                nc.tensor.matmul(
                    h_ps,
                    lhsT=w1t_sb[:, c * 128 : (c + 1) * 128],
                    rhs=xt_mm,
                    start=True,
                    stop=True,
                )
                nc.scalar.activation(
                    out=z_sb[:, c, :], in_=h_ps, func=AF.Relu,
                    bias=b1_sb[:, c : c + 1], scale=1.0,
                )
                if c in DVE_ZHI:
                    # min(h+b1, 0) = -relu(-h-b1); W2 rows for this chunk
                    # are negated host-side
                    nc.vector.tensor_scalar(
                        out=z_sb[:, 4 + c, :], in0=h_ps,
                        scalar1=b1_sb[:, c : c + 1], scalar2=0.0,
                        op0=mybir.AluOpType.add, op1=mybir.AluOpType.min,
                    )
                else:
                    nc.scalar.activation(
                        out=z_sb[:, 4 + c, :], in_=h_ps, func=AF.Relu,
                        bias=nb1_sb[:, c : c + 1], scale=-1.0,
                    )

            return rows, z_sb

        def front_b(rows, z_sb, last=False):
            # ---- linear2, batch-major: y_s [128 batch-sub, 512 leaves] ----
            # z is the stationary operand and W2T the moving one, so y comes
            # out batch-major and the segment max is a single strided
            # free-dim reduce straight off each PSUM bank — no
            # transpose-back, no partition folds.
            pl = sm.tile([128, 4, N_ACTIONS], fp32, tag="pl")
            for s in range(4):
                y_ps = psY.tile([128, B_TILE], fp32, tag="y")
                for k in range(8):
                    nc.tensor.matmul(
                        y_ps,
                        lhsT=z_sb[:, k, s * 128 : (s + 1) * 128],
                        rhs=w2t_sb[:, k, :],
                        start=(k == 0),
                        stop=(k == 7),
                    )
                nc.vector.tensor_reduce(
                    out=pl[:, s, :],
                    in_=y_ps.rearrange("p (j a) -> p a j", a=N_ACTIONS),
                    axis=mybir.AxisListType.X,
                    op=mybir.AluOpType.max,
                )
                if last:
                    # final tile: softmax+store per subtile right after its
                    # reduce, so only one subtile's chain trails the last MM
                    negmx_s = sm.tile([128, 1], fp32, tag="negmx_l")
                    nc.vector.tensor_reduce(
                        out=negmx_s, in_=pl[:, s, :], axis=mybir.AxisListType.X,
                        op=mybir.AluOpType.max, negate=True,
                    )
                    e_s = sm.tile([128, N_ACTIONS], fp32, tag="e_l")
                    ssum_s = sm.tile([128, 1], fp32, tag="ssum_l")
                    nc.scalar.activation(
                        out=e_s, in_=pl[:, s, :], func=AF.Exp,
                        bias=negmx_s, scale=1.0, accum_out=ssum_s,
                    )
                    rcp_s = sm.tile([128, 1], fp32, tag="rcp_l")
                    nc.vector.reciprocal(rcp_s, ssum_s)
                    o_s = sm.tile([128, N_ACTIONS], fp32, tag="o_l")
                    nc.vector.tensor_scalar_mul(o_s, e_s, rcp_s)
                    nc.sync.dma_start(
                        out=out_h[rows.start + s * 128 : rows.start + (s + 1) * 128, :],
                        in_=o_s,
                    )
            if last:
                return None
            return rows, pl

        def back(rows, pl):
            # ---- softmax, batch-major [128, 4, 16] ----
            negmx = sm.tile([128, 4], fp32, tag="negmx")
            nc.vector.tensor_reduce(
                out=negmx, in_=pl, axis=mybir.AxisListType.X,
                op=mybir.AluOpType.max, negate=True,
            )
            e = sm.tile([128, 4, N_ACTIONS], fp32, tag="e")
            ssum = sm.tile([128, 4], fp32, tag="ssum")
            for s in range(4):
                nc.scalar.activation(
                    out=e[:, s, :], in_=pl[:, s, :], func=AF.Exp,
                    bias=negmx[:, s : s + 1], scale=1.0,
                    accum_out=ssum[:, s : s + 1],
                )
            rcp = sm.tile([128, 4], fp32, tag="rcp")
            nc.vector.reciprocal(rcp, ssum)
            o = sm.tile([128, 4, N_ACTIONS], fp32, tag="o")
            for s in range(4):
                nc.vector.tensor_scalar_mul(o[:, s, :], e[:, s, :], rcp[:, s : s + 1])

            nc.sync.dma_start(
                out=out_h[rows, :].rearrange("(s p) a -> p s a", p=128), in_=o
            )

        # 3-deep software pipeline: front_a (x -> z) runs two tiles ahead
        # of front_b (mm2 + pooled reduce), so the scalar-engine z chunks are
        # ready long before their matmul2; back trails one tile behind.
        total = N_TILES * n_passes
        fa = [front_a(0, x_pre[0])]
        if total > 1:
            fa.append(front_a(1, x_pre[1]))
        pending = None
        for i in range(total):
            cur = front_b(*fa.pop(0), last=(i == total - 1))
            if i + 2 < total:
                fa.append(front_a((i + 2) % N_TILES))
            if pending is not None:
                back(*pending)
            pending = cur
        if pending is not None:
            back(*pending)

    nc.compile()
    return nc


def _prep_weights(W1, b1, W2, leaf_actions):
    """Host-side weight prep: pad/transpose W1, pad W2 and permute leaves so
    slot s holds a leaf of action s % 16 (round-robin over each group).
    W2 rows feeding the DVE-produced z_hi chunks (min-trick) are negated."""
    w1t = np.zeros((IN_DIM, NODES_P), np.float32)
    w1t[:, :N_NODES] = np.asarray(W1, np.float32).T
    b1c = np.zeros((4, 128), np.float32)
    b1c.reshape(-1)[:N_NODES] = np.asarray(b1, np.float32)
    b1c = np.ascontiguousarray(b1c.T)          # [128, 4]
    nb1c = np.ascontiguousarray(-b1c)

    la = np.asarray(leaf_actions).astype(np.int64)
    perm = np.empty(N_LEAVES, np.int64)
    per_action = N_LEAVES // N_ACTIONS
    for a in range(N_ACTIONS):
        (grp,) = np.nonzero(la == a)
        assert len(grp) == per_action, "kernel assumes 32 leaves per action"
        perm[a + N_ACTIONS * np.arange(per_action)] = grp

    W2 = np.asarray(W2, np.float32)[perm]       # [512, 1022] leaf-permuted
    w2t = np.zeros((Z_DIM, N_LEAVES), np.float32)
    w2t[:N_NODES, :] = W2[:, :N_NODES].T        # relu(h) half
    w2t[NODES_P : NODES_P + N_NODES, :] = W2[:, N_NODES:].T  # relu(-h) half
    for c in DVE_ZHI:                           # min-trick chunks: z negated
        w2t[NODES_P + c * 128 : NODES_P + (c + 1) * 128, :] *= -1.0
    w2t = np.ascontiguousarray(
        w2t.reshape(8, 128, N_LEAVES).transpose(1, 0, 2)
    )                                           # [128, 8, 512]
    if MM2_BF16:
        import ml_dtypes
        w2t = w2t.astype(ml_dtypes.bfloat16)
    return w1t, b1c, nb1c, w2t


_runner = None  # (jitted shard_map fn, in_names, zeros) — persists across calls


def _make_runner(nc):
    """Jitted shard_map wrapper around the bass_exec custom call (mirrors
    bass2jax.run_bass_via_pjrt's multi-core path, but reusable across calls
    so the NEFF is compiled once per process)."""
    import jax
    import numpy as _np
    from jax.sharding import Mesh, PartitionSpec, NamedSharding
    from jax.experimental.shard_map import shard_map
    from concourse import bass2jax, mybir

    bass2jax.install_neuronx_cc_hook()
    partition_name = nc.partition_id_tensor.name if nc.partition_id_tensor else None
    in_names, out_names, out_avals, zero_shapes = [], [], [], []
    for alloc in nc.m.functions[0].allocations:
        if not isinstance(alloc, mybir.MemoryLocationSet):
            continue
        name = alloc.memorylocations[0].name
        if alloc.kind == "ExternalInput":
            if name != partition_name:
                in_names.append(name)
        elif alloc.kind == "ExternalOutput":
            shape = tuple(alloc.tensor_shape)
            dtype = mybir.dt.np(alloc.dtype)
            out_names.append(name)
            out_avals.append(jax.core.ShapedArray(shape, dtype))
            zero_shapes.append((shape, dtype))
    n_params = len(in_names)
    all_in_names = in_names + out_names + ([partition_name] if partition_name else [])

    def _body(*args):
        operands = list(args)
        if partition_name is not None:
            operands.append(bass2jax.partition_id_tensor())
        return tuple(bass2jax._bass_exec_p.bind(
            *operands, out_avals=tuple(out_avals), in_names=tuple(all_in_names),
            out_names=tuple(out_names), lowering_input_output_aliases=(),
            sim_require_finite=True, sim_require_nnan=True, nc=nc))

    mesh = Mesh(_np.asarray(jax.devices()[:N_CORES]), ("core",))
    spec = PartitionSpec("core")
    n_outs = len(out_names)
    fn = jax.jit(
        shard_map(_body, mesh=mesh, in_specs=(spec,) * (n_params + n_outs),
                  out_specs=(spec,) * n_outs, check_rep=False),
        keep_unused=True)
    sh = NamedSharding(mesh, spec)
    zeros = tuple(
        jax.device_put(_np.zeros((N_CORES * s[0], *s[1:]), d), sh)
        for s, d in zero_shapes)
    return fn, in_names, sh, zeros


def kernel(x, W1, b1, W2, leaf_actions):
    global _compiled, _runner
    import jax

    x = np.ascontiguousarray(np.asarray(x, np.float32))
    assert x.shape == (B, IN_DIM)
    w1t, b1c, nb1c, w2t = _prep_weights(W1, b1, W2, leaf_actions)

    if _compiled is None:
        _compiled = _build_nc()
    if _runner is None:
        _runner = _make_runner(_compiled)
    fn, in_names, sh, zeros = _runner

    full = {"x": x, "w1t": np.concatenate([w1t] * N_CORES, axis=0),
            "b1c": np.concatenate([b1c] * N_CORES, axis=0),
            "nb1c": np.concatenate([nb1c] * N_CORES, axis=0),
            "w2t": np.concatenate([w2t] * N_CORES, axis=0)}
    dev_in = [jax.device_put(full[nm], sh) for nm in in_names]
    out = fn(*dev_in, *zeros)
    return np.asarray(out[0])



# revision 2
# speedup vs baseline: 2.8818x; 2.8818x over previous
"""DTNetv0 forward kernel for 8 Trainium2 NeuronCores.

Computes, for x [B,128], W1 [511,128], b1 [511], W2 [512,1022],
leaf_actions [512] (32 leaves per each of 16 actions):

    h = x @ W1.T + b1
    z = [relu(h), relu(-h)]
    y = z @ W2.T
    pooled[b,a] = max over leaves l with action a of y[b,l]
    out = softmax(pooled, axis=-1)

Sharding: pure data parallelism — batch split 8 ways, weights replicated.

Algebraic fold (the key PE-cycle saver): with W2 = [W2a | W2b] split at the
relu(h)/relu(-h) boundary and relu(h) = (h+|h|)/2, relu(-h) = (|h|-h)/2,

    y = h @ A.T + |h| @ B.T,   A = (W2a - W2b)/2,  B = (W2a + W2b)/2
      = x @ C.T + |h| @ B.T + c0,   C = A @ W1,  c0 = A @ b1

so the 1022-wide second contraction becomes a 511-wide one (|h| @ B.T) plus
a 128-wide one (x @ C.T) — 10240 PE cycles/tile for linear2 instead of
16384, and the ACT engine computes one Abs per node instead of two Relus.
c0 rides for free through the padded node row: w1t column 511 is zero and
b1 pad is 1.0, so |h|[511] == 1 exactly, and B.T row 511 holds c0.

Per 512-row batch tile, on device (x arrives HOST-TRANSPOSED as
xt [128 in, B_SHARD], so there are no PE transposes at all):
    hT [512nodes,512b] linear1: 4 f32r matmuls (W1T stationary, xt moving)
    aT [512, 512b]     Abs(h + b1) on ACT, straight off each PSUM bank
    y  [128b, 512lv]   linear2 BATCH-MAJOR: per 128-batch subtile, 1 matmul
                       with the xt chunk stationary and C.T moving, then 4
                       accumulating matmuls with the aT chunk stationary and
                       B.T moving — y lands batch-major in PSUM
    pooled [128b, 16]  leaves are host-permuted so slot s holds a leaf of
                       action s%16; the whole segment max is ONE strided DVE
                       reduce straight off each PSUM bank
    out                softmax batch-major: Exp with per-partition bias=-max
                       and accum_out for the denominator, reciprocal, scale

Three-stage software pipeline in emission order: front_a (x -> aT) runs two
tiles ahead of front_b (linear2 + pooled reduce), and the softmax tail
trails one tile behind, so the aT chunks PE needs are always ready.
Steady state is 12288 PE cycles/tile (2048 mm1 + 2048 x-fold + 8192 abs),
~5.1 us/tile, PE-bound.

Matmul operands use float32r: fp32 data processed at 1 cycle/row for
512-wide moving operands (plain fp32 runs at 4 cycles/row). bf16 is
row-rate-parity with f32r on the PE, and fp8's quantization noise exceeds
the max-err budget — f32r everywhere.
"""

import numpy as np

B, IN_DIM, N_NODES, N_LEAVES, N_ACTIONS = 131072, 128, 511, 512, 16
N_CORES = 8
B_SHARD = B // N_CORES          # 16384 rows per core
B_TILE = 512                    # batch columns per tile (one PSUM bank of fp32)
N_TILES = B_SHARD // B_TILE     # 32
NODES_P = 512                   # nodes padded 511 -> 512 (4 chunks of 128)

_compiled = None  # traced+compiled Bass module cache (one per process)


def _build_nc(n_passes=1):
    import concourse.tile as tile
    from concourse import bacc, mybir
    from contextlib import ExitStack

    fp32 = mybir.dt.float32
    f32r = mybir.dt.float32r
    AF = mybir.ActivationFunctionType

    nc = bacc.Bacc()
    xt_h = nc.declare_dram_parameter("xt", [IN_DIM, B_SHARD], f32r, isOutput=False)
    w1t_h = nc.declare_dram_parameter("w1t", [IN_DIM, NODES_P], f32r, isOutput=False)
    b1c_h = nc.declare_dram_parameter("b1c", [128, 4], fp32, isOutput=False)
    bt_h = nc.declare_dram_parameter("bt", [128, 4, N_LEAVES], f32r, isOutput=False)
    ct_h = nc.declare_dram_parameter("ct", [IN_DIM, N_LEAVES], f32r, isOutput=False)
    out_h = nc.declare_dram_parameter("out", [B_SHARD, N_ACTIONS], fp32, isOutput=True)

    with tile.TileContext(nc) as tc, ExitStack() as ctx:
        consts = ctx.enter_context(tc.tile_pool(name="consts", bufs=1))
        xin = ctx.enter_context(tc.tile_pool(name="xin", bufs=4))
        ap = ctx.enter_context(tc.tile_pool(name="ap", bufs=3))
        sm = ctx.enter_context(tc.tile_pool(name="sm", bufs=2))
        psA = ctx.enter_context(tc.tile_pool(name="psA", bufs=2, space="PSUM"))
        psY = ctx.enter_context(tc.tile_pool(name="psY", bufs=5, space="PSUM"))

        def load_x(t):
            x_sb = xin.tile([128, B_TILE], f32r, tag="x")
            nc.sync.dma_start(out=x_sb, in_=xt_h[:, t * B_TILE : (t + 1) * B_TILE])
            return x_sb

        # prefetch the first two x tiles before the (big) weight DMAs so the
        # first linear1 matmuls are not queued behind them
        x_pre = [load_x(0), load_x(1)]
        b1_sb = consts.tile([128, 4], fp32)
        nc.sync.dma_start(out=b1_sb, in_=b1c_h[:, :])
        # weights are declared float32r in DRAM (host sends fp32 bits) and
        # DMA straight into f32r tiles. The 1MB bt load rides the Activation
        # HWDGE queue so the x-loads (SP queue) are not stuck behind it.
        w1t_sb = consts.tile([128, NODES_P], f32r)
        nc.sync.dma_start(out=w1t_sb, in_=w1t_h[:, :])
        ct_sb = consts.tile([128, N_LEAVES], f32r)
        nc.sync.dma_start(out=ct_sb, in_=ct_h[:, :])
        bt_sb = consts.tile([128, 4, N_LEAVES], f32r)
        nc.scalar.dma_start(out=bt_sb, in_=bt_h[:, :, :])

        def front_a(t, x_sb=None):
            rows = slice(t * B_TILE, (t + 1) * B_TILE)
            if x_sb is None:
                x_sb = load_x(t)

            # ---- linear1 + fused bias/abs into aT [128, 4, 512] ----
            a_sb = ap.tile([128, 4, B_TILE], f32r, tag="a")
            for c in range(4):
                h_ps = psA.tile([128, B_TILE], fp32, tag="h")
                nc.tensor.matmul(
                    h_ps,
                    lhsT=w1t_sb[:, c * 128 : (c + 1) * 128],
                    rhs=x_sb,
                    start=True,
                    stop=True,
                )
                nc.scalar.activation(
                    out=a_sb[:, c, :], in_=h_ps, func=AF.Abs,
                    bias=b1_sb[:, c : c + 1], scale=1.0,
                )
            return rows, x_sb, a_sb

        def front_b(rows, x_sb, a_sb, last=False):
            # ---- linear2, batch-major: y_s [128 batch-sub, 512 leaves] ----
            # per subtile: x chunk stationary vs C.T moving (the folded
            # linear part, 128-wide contraction), then the 4 aT chunks
            # stationary vs B.T moving. y comes out batch-major so the
            # segment max is a single strided free-dim reduce straight off
            # each PSUM bank — no transpose-back, no partition folds.
            pl = sm.tile([128, 4, N_ACTIONS], fp32, tag="pl")
            for s in range(4):
                y_ps = psY.tile([128, B_TILE], fp32, tag="y")
                nc.tensor.matmul(
                    y_ps,
                    lhsT=x_sb[:, s * 128 : (s + 1) * 128],
                    rhs=ct_sb,
                    start=True,
                    stop=False,
                )
                for c in range(4):
                    nc.tensor.matmul(
                        y_ps,
                        lhsT=a_sb[:, c, s * 128 : (s + 1) * 128],
                        rhs=bt_sb[:, c, :],
                        start=False,
                        stop=(c == 3),
                    )
                nc.vector.tensor_reduce(
                    out=pl[:, s, :],
                    in_=y_ps.rearrange("p (j a) -> p a j", a=N_ACTIONS),
                    axis=mybir.AxisListType.X,
                    op=mybir.AluOpType.max,
                )
                if last:
                    # final tile: softmax+store per subtile right after its
                    # reduce, so only one subtile's chain trails the last MM
                    negmx_s = sm.tile([128, 1], fp32, tag="negmx_l")
                    nc.vector.tensor_reduce(
                        out=negmx_s, in_=pl[:, s, :], axis=mybir.AxisListType.X,
                        op=mybir.AluOpType.max, negate=True,
                    )
                    e_s = sm.tile([128, N_ACTIONS], fp32, tag="e_l")
                    ssum_s = sm.tile([128, 1], fp32, tag="ssum_l")
                    nc.scalar.activation(
                        out=e_s, in_=pl[:, s, :], func=AF.Exp,
                        bias=negmx_s, scale=1.0, accum_out=ssum_s,
                    )
                    rcp_s = sm.tile([128, 1], fp32, tag="rcp_l")
                    nc.vector.reciprocal(rcp_s, ssum_s)
                    o_s = sm.tile([128, N_ACTIONS], fp32, tag="o_l")
                    nc.vector.tensor_scalar_mul(o_s, e_s, rcp_s)
                    nc.sync.dma_start(
                        out=out_h[rows.start + s * 128 : rows.start + (s + 1) * 128, :],
                        in_=o_s,
                    )
            if last:
                return None
            return rows, pl

        def back(rows, pl):
            # ---- softmax, batch-major [128, 4, 16] ----
            negmx = sm.tile([128, 4], fp32, tag="negmx")
            nc.vector.tensor_reduce(
                out=negmx, in_=pl, axis=mybir.AxisListType.X,
                op=mybir.AluOpType.max, negate=True,
            )
            e = sm.tile([128, 4, N_ACTIONS], fp32, tag="e")
            ssum = sm.tile([128, 4], fp32, tag="ssum")
            for s in range(4):
                nc.scalar.activation(
                    out=e[:, s, :], in_=pl[:, s, :], func=AF.Exp,
                    bias=negmx[:, s : s + 1], scale=1.0,
                    accum_out=ssum[:, s : s + 1],
                )
            rcp = sm.tile([128, 4], fp32, tag="rcp")
            nc.vector.reciprocal(rcp, ssum)
            o = sm.tile([128, 4, N_ACTIONS], fp32, tag="o")
            for s in range(4):
                nc.vector.tensor_scalar_mul(o[:, s, :], e[:, s, :], rcp[:, s : s + 1])

            nc.sync.dma_start(
                out=out_h[rows, :].rearrange("(s p) a -> p s a", p=128), in_=o
            )

        # 3-deep software pipeline: front_a (x -> aT) runs two tiles ahead
        # of front_b (linear2 + pooled reduce), so the ACT-engine aT chunks
        # are ready long before their matmuls; back trails one tile behind.
        total = N_TILES * n_passes
        fa = [front_a(0, x_pre[0])]
        if total > 1:
            fa.append(front_a(1, x_pre[1]))
        pending = None
        for i in range(total):
            cur = front_b(*fa.pop(0), last=(i == total - 1))
            if i + 2 < total:
                fa.append(front_a((i + 2) % N_TILES))
            if pending is not None:
                back(*pending)
            pending = cur
        if pending is not None:
            back(*pending)

    nc.compile()
    return nc


def _prep_weights(W1, b1, W2, leaf_actions):
    """Host-side weight prep: fold the linear half of the relu pair into x
    (C = A@W1, c0 = A@b1) and keep only the |h| half (B) at full width.
    Leaves are permuted so slot s holds a leaf of action s % 16."""
    W1 = np.asarray(W1, np.float64)
    b1 = np.asarray(b1, np.float64)
    W2 = np.asarray(W2, np.float64)

    la = np.asarray(leaf_actions).astype(np.int64)
    perm = np.empty(N_LEAVES, np.int64)
    per_action = N_LEAVES // N_ACTIONS
    for a in range(N_ACTIONS):
        (grp,) = np.nonzero(la == a)
        assert len(grp) == per_action, "kernel assumes 32 leaves per action"
        perm[a + N_ACTIONS * np.arange(per_action)] = grp

    W2p = W2[perm]                              # [512, 1022] leaf-permuted
    Am = (W2p[:, :N_NODES] - W2p[:, N_NODES:]) * 0.5   # [512, 511]
    Bm = (W2p[:, :N_NODES] + W2p[:, N_NODES:]) * 0.5   # [512, 511]
    C = Am @ W1                                 # [512, 128]
    c0 = Am @ b1                                # [512]

    w1t = np.zeros((IN_DIM, NODES_P), np.float32)
    w1t[:, :N_NODES] = W1.T                     # col 511 stays zero
    b1c = np.zeros((4, 128), np.float32)
    b1c.reshape(-1)[:N_NODES] = b1
    b1c.reshape(-1)[N_NODES] = 1.0              # pad node: |h|[511] == 1
    b1c = np.ascontiguousarray(b1c.T)           # [128, 4]

    btm = np.zeros((NODES_P, N_LEAVES), np.float32)
    btm[:N_NODES, :] = Bm.T
    btm[N_NODES, :] = c0                        # bias row rides the pad node
    bt = np.ascontiguousarray(
        btm.reshape(4, 128, N_LEAVES).transpose(1, 0, 2)
    )                                           # [128, 4, 512]
    ct = np.ascontiguousarray(C.T.astype(np.float32))  # [128, 512]
    return w1t, b1c, bt, ct


_runner = None  # (jitted shard_map fn, in_names, zeros) — persists across calls


def _make_runner(nc):
    """Jitted shard_map wrapper around the bass_exec custom call (mirrors
    bass2jax.run_bass_via_pjrt's multi-core path, but reusable across calls
    so the NEFF is compiled once per process)."""
    import jax
    import numpy as _np
    from jax.sharding import Mesh, PartitionSpec, NamedSharding
    from jax.experimental.shard_map import shard_map
    from concourse import bass2jax, mybir

    bass2jax.install_neuronx_cc_hook()
    partition_name = nc.partition_id_tensor.name if nc.partition_id_tensor else None
    in_names, out_names, out_avals, zero_shapes = [], [], [], []
    for alloc in nc.m.functions[0].allocations:
        if not isinstance(alloc, mybir.MemoryLocationSet):
            continue
        name = alloc.memorylocations[0].name
        if alloc.kind == "ExternalInput":
            if name != partition_name:
                in_names.append(name)
        elif alloc.kind == "ExternalOutput":
            shape = tuple(alloc.tensor_shape)
            dtype = mybir.dt.np(alloc.dtype)
            out_names.append(name)
            out_avals.append(jax.core.ShapedArray(shape, dtype))
            zero_shapes.append((shape, dtype))
    n_params = len(in_names)
    all_in_names = in_names + out_names + ([partition_name] if partition_name else [])

    def _body(*args):
        operands = list(args)
        if partition_name is not None:
            operands.append(bass2jax.partition_id_tensor())
        return tuple(bass2jax._bass_exec_p.bind(
            *operands, out_avals=tuple(out_avals), in_names=tuple(all_in_names),
            out_names=tuple(out_names), lowering_input_output_aliases=(),
            sim_require_finite=True, sim_require_nnan=True, nc=nc))

    mesh = Mesh(_np.asarray(jax.devices()[:N_CORES]), ("core",))
    spec = PartitionSpec("core")
    n_outs = len(out_names)
    fn = jax.jit(
        shard_map(_body, mesh=mesh, in_specs=(spec,) * (n_params + n_outs),
                  out_specs=(spec,) * n_outs, check_rep=False),
        keep_unused=True)
    sh = NamedSharding(mesh, spec)
    zeros = tuple(
        jax.device_put(_np.zeros((N_CORES * s[0], *s[1:]), d), sh)
        for s, d in zero_shapes)
    return fn, in_names, sh, zeros


def kernel(x, W1, b1, W2, leaf_actions):
    global _compiled, _runner
    import jax

    x = np.ascontiguousarray(np.asarray(x, np.float32))
    assert x.shape == (B, IN_DIM)
    w1t, b1c, bt, ct = _prep_weights(W1, b1, W2, leaf_actions)
    # host-side shard + transpose: core i gets x[i*B_SHARD:(i+1)*B_SHARD].T
    xt = np.ascontiguousarray(
        x.reshape(N_CORES, B_SHARD, IN_DIM).transpose(0, 2, 1)
    ).reshape(N_CORES * IN_DIM, B_SHARD)

    if _compiled is None:
        _compiled = _build_nc()
    if _runner is None:
        _runner = _make_runner(_compiled)
    fn, in_names, sh, zeros = _runner

    full = {"xt": xt, "w1t": np.concatenate([w1t] * N_CORES, axis=0),
            "b1c": np.concatenate([b1c] * N_CORES, axis=0),
            "bt": np.concatenate([bt] * N_CORES, axis=0),
            "ct": np.concatenate([ct] * N_CORES, axis=0)}
    dev_in = [jax.device_put(full[nm], sh) for nm in in_names]
    out = fn(*dev_in, *zeros)
    return np.asarray(out[0])
